# revision 31
# baseline (speedup 1.0000x reference)
"""Trainium2 Bass kernel for the Ewald real-space potential (N=2048, 8 cores).

Strategy (1D row-parallel, hint-compliant):
  - Core k owns output rows i in [256k, 256k+256). All O(N^2) work on device.
  - Per-core layout: j on SBUF partitions (16 tiles of 128), i on the free dim
    (256 columns). Everything per-pair is a function of rsq[j,i] plus low-rank
    bilinear factors, so:
      * rsq comes from ONE rank-5 fp32 matmul per j-tile:
          rsq = g_j + g_i - 2 r_j.r_i
      * all reciprocal powers come from exp/ln ACT evals (rsqrt/recip are
        banned on ACT):  V=1/r, W=1/r^2, rnorm=r, plus gauss exponentials
        with constants folded into exp's bias.
      * three bounded per-pair fields are built on DVE:
          F0 = erf(a r)/r                       (fp32, charge-charge)
          s1 = (F0 - c*gauss) / r^2             (bf16 out, charge-dipole)
          H  = s2 / r^2                         (bf16 out, dipole-dipole)
        where s2 = (2*SA + F0)/r^2 - 2a^2 c gauss,  SA = F0 - c*gauss.
      * the j-sums are PE matmuls of the fields against a 20-column weight
        matrix  w_j = [q, u, e, q*r, e*r, r (x) u]  accumulated over j-tiles
        in PSUM. F0 reduces in fp32 (the qq term has ~1400x cancellation
        amplification; bf16 there costs 4e-3 rel err), s1/H reduce in bf16
        (measured error contribution ~1e-4).
  - The diagonal j==i is killed by adding 1e20 to rsq there. To keep the
    program SPMD-identical across cores, the host permutes each core's
    j-tiles so the two diagonal-containing tiles are always slots 0 and 1.
  - O(N) epilogue (assembling per-i dot products from the 41 weighted sums
    and the final scalar) runs on host in f64, as does the 8-way partial sum.
"""

import os
import numpy as np
import ml_dtypes

import concourse.bass as bass
import concourse.bacc as bacc
import concourse.tile as tile
from concourse import mybir
from concourse import bass_utils

N = 2048
NCORES = 8
RPC = N // NCORES          # 256 rows (i) per core
NT = N // 128              # 16 j-tiles
SIGMA = 1.0
A = 1.0 / (SIGMA * np.sqrt(2.0))
C = 2.0 * A / np.sqrt(np.pi)
NORM = 90.0474
BIG = np.float32(1e20)
RSQ_EPS = 1e-3

F32 = mybir.dt.float32
BF16 = mybir.dt.bfloat16

TRACE = bool(int(os.environ.get("KERNEL_TRACE", "0")))
LAST_EXEC_NS = None
LAST_TRACE = None

_NC_CACHE = {}


def _ensure_ntff_hook():
    """Register the axon NTFF profile hook if the image's antenv lacks it,
    and neuter the S3 artifact upload. Trace-only; never on the grading path."""
    import sys
    import types

    try:
        from antenv.axon_hooks import get_axon_ntff_profile_hook  # noqa: F401
        registered = get_axon_ntff_profile_hook() is not None
    except ImportError:
        import antenv

        mod = types.ModuleType("antenv.axon_hooks")
        _h = {"hook": None}
        mod.set_axon_ntff_profile_hook = lambda h: _h.__setitem__("hook", h)
        mod.get_axon_ntff_profile_hook = lambda: _h["hook"]
        sys.modules["antenv.axon_hooks"] = mod
        antenv.axon_hooks = mod
        registered = False
    if not registered:
        from antenv.axon_hooks import set_axon_ntff_profile_hook

        if "/root/.axon_site" not in sys.path:
            sys.path.insert(0, "/root/.axon_site")
        from trn_agent_boot.trn_boot import _ntff_profile_via_ctypes

        set_axon_ntff_profile_hook(
            _ntff_profile_via_ctypes("/opt/axon/libaxon_pjrt.so")
        )
    # no S3 in this container — keep artifacts local
    bass_utils.upload_artifacts = lambda tmpdir: tmpdir


def _build_nc():
    from contextlib import ExitStack

    nc = bacc.Bacc("TRN2", target_bir_lowering=False)

    # aug_l (cols 0:N) and aug_ri (cols N:N+RPC) packed in one tensor so the
    # first rsq matmul waits on a single DMA semaphore (fp32 self-loading
    # matmuls can't carry two waits through walrus' LDWEIGHTS codegen).
    aug_d = nc.dram_tensor("aug", [5, N + RPC], F32, kind="ExternalInput")
    wtq_d = nc.dram_tensor("wtq", [128, NT], F32, kind="ExternalInput")
    wt20_d = nc.dram_tensor("wt20", [128, NT, 20], BF16, kind="ExternalInput")
    wt20f_d = nc.dram_tensor("wt20f", [128, NT, 20], F32, kind="ExternalInput")
    # [BIG*eye | eye]: the diagonal bump is applied by an accumulating PE
    # matmul (BIG*I @ I adds BIG on the diagonal), avoiding a DVE op on PSUM
    # that would need more sync waits than walrus codegen allows.
    eyes_d = nc.dram_tensor("eyes", [128, 256], F32, kind="ExternalInput")

    accq_d = nc.dram_tensor("accq", [1, RPC], F32, kind="ExternalOutput")
    accs_d = nc.dram_tensor("accs", [20, RPC], F32, kind="ExternalOutput")
    acch_d = nc.dram_tensor("acch", [20, RPC], F32, kind="ExternalOutput")

    neg_a2 = float(-A * A)
    ln_c = float(np.log(C))
    ln_2a2c = float(np.log(2.0 * A * A * C))

    with tile.TileContext(nc) as tc, ExitStack() as ctx:
        singles = ctx.enter_context(tc.tile_pool(name="singles", bufs=1))
        ff = ctx.enter_context(tc.tile_pool(name="ff", bufs=7))
        fb = ctx.enter_context(tc.tile_pool(name="fb", bufs=2))

        # ---- load inputs ----
        sb_aug = singles.tile([5, N + RPC], F32)
        nc.sync.dma_start(out=sb_aug, in_=aug_d[:, :])
        sb_augl = sb_aug[:, 0:N]
        sb_augri = sb_aug[:, N:N + RPC]
        sb_wtq = singles.tile([128, NT], F32)
        nc.sync.dma_start(out=sb_wtq, in_=wtq_d[:, :])
        sb_wt20 = singles.tile([128, NT, 20], BF16)
        nc.sync.dma_start(out=sb_wt20, in_=wt20_d[:, :, :])
        sb_wt20f = singles.tile([128, NT, 20], F32)
        nc.sync.dma_start(out=sb_wt20f, in_=wt20f_d[:, :, :])
        sb_eyes = singles.tile([128, 256], F32)
        nc.sync.dma_start(out=sb_eyes, in_=eyes_d[:, :])

        # bias constants for the exp evals (activation bias must be an AP)
        b_lnc = singles.tile([128, 1], F32)
        nc.vector.memset(b_lnc, ln_c)
        b_ln2a2c = singles.tile([128, 1], F32)
        nc.vector.memset(b_ln2a2c, ln_2a2c)

        # ---- rsq in PSUM: scoped pool so the banks free before accumulators ----
        L = ff.tile([128, N * 2], F32, tag="ffield")
        cG = ff.tile([128, N * 2], F32, tag="ffield")
        t2G = ff.tile([128, N * 2], F32, tag="ffield")
        with tc.tile_pool(name="psum_rsq", bufs=1, space="PSUM") as psum_rsq:
            rsq = psum_rsq.tile([128, NT * RPC], F32)  # [128, 4096] = all 8 banks
            for t in range(NT):
                # two 256-col tiles share each 2KB bank; start=True clears
                # has_written for the WHOLE bank, so only the bank's first
                # matmul may use it (else the later diag-bump accumulation
                # silently degrades to an overwrite).
                nc.tensor.matmul(
                    rsq[:, t * RPC:(t + 1) * RPC],
                    lhsT=sb_augl[:, t * 128:(t + 1) * 128],
                    rhs=sb_augri,
                    start=(t % 2 == 0),
                    stop=True,
                    skip_group_check=True,
                )
            # kill the diagonal (always in permuted slots 0,1): accumulate
            # BIG*I @ I onto the two 128-col diagonal regions
            for off in (0, 384):
                nc.tensor.matmul(
                    rsq[:, off:off + 128],
                    lhsT=sb_eyes[:, 0:128],
                    rhs=sb_eyes[:, 128:256],
                    start=False,
                    stop=True,
                    skip_group_check=True,
                )

            # ---- ACT set 1: natural_log_exp (rsq readers) ----
            nc.scalar.activation(L, rsq, mybir.ActivationFunctionType.Ln)
            nc.scalar.activation(
                cG, rsq, mybir.ActivationFunctionType.Exp, scale=neg_a2, bias=b_lnc
            )
            nc.scalar.activation(
                t2G, rsq, mybir.ActivationFunctionType.Exp, scale=neg_a2, bias=b_ln2a2c
            )

        V = ff.tile([128, N * 2], F32, tag="ffield")
        nc.scalar.activation(V, L, mybir.ActivationFunctionType.Exp, scale=-0.5)
        W = ff.tile([128, N * 2], F32, tag="ffield")
        nc.scalar.activation(W, L, mybir.ActivationFunctionType.Exp, scale=-1.0)
        rnorm = ff.tile([128, N * 2], F32, tag="ffield")
        nc.scalar.activation(rnorm, L, mybir.ActivationFunctionType.Exp, scale=0.5)
        # ---- ACT set 2: sigmoid (erf) ----
        E = ff.tile([128, N * 2], F32, tag="ffield")
        nc.scalar.activation(E, rnorm, mybir.ActivationFunctionType.Erf, scale=float(A))

        # ---- DVE field chain (fp32; bf16 only on final field writes) ----
        F0 = ff.tile([128, N * 2], F32, tag="ffield")
        nc.vector.tensor_mul(F0, E, V)
        SA = ff.tile([128, N * 2], F32, tag="ffield")
        nc.vector.tensor_sub(SA, F0, cG)
        s1b = fb.tile([128, N * 2], BF16, tag="bfield")
        nc.vector.tensor_mul(s1b, SA, W)
        SS = ff.tile([128, N * 2], F32, tag="ffield")
        nc.vector.scalar_tensor_tensor(
            SS, SA, 2.0, F0, op0=mybir.AluOpType.mult, op1=mybir.AluOpType.add
        )
        z = ff.tile([128, N * 2], F32, tag="ffield")
        nc.vector.tensor_mul(z, SS, W)
        z2 = ff.tile([128, N * 2], F32, tag="ffield")
        nc.vector.tensor_sub(z2, z, t2G)
        # H is unbounded (~1/rsq^2, up to ~5e4 on the closest pair) and its
        # reduce-side bilinear expansion cancels heavily, so H must stay fp32
        # (bf16 H costs ~3e-2 relative on the total).
        Hf = ff.tile([128, N * 2], F32, tag="ffield")
        nc.vector.tensor_mul(Hf, z2, W)

        # ---- PE weighted j-reductions, PSUM-accumulated over tiles ----
        psum_acc = ctx.enter_context(tc.tile_pool(name="pacc", bufs=1, space="PSUM"))
        accq = psum_acc.tile([1, RPC], F32)
        accs = psum_acc.tile([20, RPC], F32)
        acch = psum_acc.tile([20, RPC], F32)
        for t in range(NT):
            sl = slice(t * RPC, (t + 1) * RPC)
            nc.tensor.matmul(
                accq,
                lhsT=sb_wtq[:, t:t + 1],
                rhs=F0[:, sl],
                start=(t == 0),
                stop=(t == NT - 1),
            )
            nc.tensor.matmul(
                accs,
                lhsT=sb_wt20[:, t, :],
                rhs=s1b[:, sl],
                start=(t == 0),
                stop=(t == NT - 1),
            )
            nc.tensor.matmul(
                acch,
                lhsT=sb_wt20f[:, t, :],
                rhs=Hf[:, sl],
                start=(t == 0),
                stop=(t == NT - 1),
            )

        # PSUM cannot be DMA'd directly; stage through SBUF
        sb_accq = singles.tile([1, RPC], F32)
        nc.vector.tensor_copy(sb_accq, accq)
        sb_accs = singles.tile([20, RPC], F32)
        nc.vector.tensor_copy(sb_accs, accs)
        sb_acch = singles.tile([20, RPC], F32)
        nc.vector.tensor_copy(sb_acch, acch)
        nc.sync.dma_start(out=accq_d[:, :], in_=sb_accq)
        nc.sync.dma_start(out=accs_d[:, :], in_=sb_accs)
        nc.sync.dma_start(out=acch_d[:, :], in_=sb_acch)

    nc.finalize()
    return nc


def _prep_inputs(q, r, u):
    """Host-side O(N) prep: per-core input dicts."""
    q64 = np.asarray(q, dtype=np.float64)
    # Centering r shrinks the fp32 rsq-matmul magnitudes ~4x (rsq and all
    # A/B/e decompositions are translation-invariant when applied uniformly).
    r64 = np.asarray(r, dtype=np.float64)
    r64 = r64 - r64.mean(0)
    u64 = np.asarray(u, dtype=np.float64)
    e64 = np.sum(r64 * u64, -1)
    g64 = np.sum(r64 * r64, -1)

    # weight matrix [N, 20]
    wt = np.zeros((N, 20), np.float64)
    wt[:, 0] = q64
    wt[:, 1:4] = u64
    wt[:, 4] = e64
    wt[:, 5:8] = q64[:, None] * r64
    wt[:, 8:11] = e64[:, None] * r64
    for cc in range(3):
        for d in range(3):
            wt[:, 11 + 3 * cc + d] = r64[:, cc] * u64[:, d]
    wt32 = wt.astype(np.float32)
    wtbf = wt32.astype(ml_dtypes.bfloat16)

    # aug rows for the rsq matmul
    augl_full = np.zeros((5, N), np.float64)
    augl_full[0:3] = -2.0 * r64.T
    augl_full[3] = 1.0
    augl_full[4] = g64

    eyes = np.zeros((128, 256), np.float32)
    p = np.arange(128)
    eyes[p, p] = BIG
    eyes[p, 128 + p] = 1.0

    in_maps = []
    percore_host = []
    for k in range(NCORES):
        order = [2 * k, 2 * k + 1] + [t for t in range(NT) if t not in (2 * k, 2 * k + 1)]
        jidx = np.concatenate([np.arange(t * 128, (t + 1) * 128) for t in order])
        i_idx = np.arange(k * RPC, (k + 1) * RPC)

        aug_ri = np.zeros((5, RPC), np.float64)
        aug_ri[0:3] = r64[i_idx].T
        # +EPS keeps rsq strictly positive: the fp32 matmul's absolute
        # rounding error (~3e-4) can exceed the true min pair rsq (~7e-6),
        # and Ln(<=0) would poison everything. All fields are smooth in rsq
        # near 0, so the shift costs ~1e-4 relative on close pairs only.
        aug_ri[3] = g64[i_idx] + RSQ_EPS
        aug_ri[4] = 1.0

        # [p, t] / [p, t, col] layouts for lhsT slicing
        wtq_pt = wt32[jidx, 0].reshape(NT, 128).T.copy()            # [128, NT]
        wt20_pt = np.ascontiguousarray(
            wtbf[jidx].reshape(NT, 128, 20).transpose(1, 0, 2)
        )                                                            # [128, NT, 20]
        wt20f_pt = np.ascontiguousarray(
            wt32[jidx].reshape(NT, 128, 20).transpose(1, 0, 2)
        )

        aug = np.concatenate([augl_full[:, jidx], aug_ri], axis=1).astype(np.float32)
        in_maps.append(
            {
                "aug": np.ascontiguousarray(aug),
                "wtq": wtq_pt,
                "wt20": wt20_pt,
                "wt20f": wt20f_pt,
                "eyes": eyes,
            }
        )
        percore_host.append(
            {
                "qi": q64[i_idx],
                "ei": e64[i_idx],
                "ri": r64[i_idx],
                "ui": u64[i_idx],
            }
        )
    return in_maps, percore_host


def _assemble(res, host):
    """Host O(N) epilogue for one core: per-i terms from the 41 weighted sums."""
    accq = res["accq"].astype(np.float64)[0]          # [256]
    accs = res["accs"].astype(np.float64)             # [20, 256]
    acch = res["acch"].astype(np.float64)
    qi, ei, ri, ui = host["qi"], host["ei"], host["ri"], host["ui"]

    T1 = accq
    V3 = accs[1:4]
    W1 = accs[4]
    X3 = accs[5:8]
    Y = accs[0]
    T2 = W1 - np.einsum("ic,ci->i", ri, V3)
    T3 = np.einsum("ic,ci->i", ui, V3)
    T4 = np.einsum("ic,ci->i", ui, X3) - ei * Y
    HU = acch[1:4]
    HE = acch[4]
    HER = acch[8:11]
    HRU = acch[11:20].reshape(3, 3, RPC)
    T5 = (
        np.einsum("ic,ci->i", ui, HER)
        - np.einsum("ic,id,cdi->i", ui, ri, HRU)
        - ei * HE
        + ei * np.einsum("ic,ci->i", ri, HU)
    )
    E_i = qi * (T1 + T2) + T3 - T4 - T5
    return E_i.sum()


def kernel(q, r, u):
    global LAST_EXEC_NS, LAST_TRACE
    q = np.asarray(q, dtype=np.float32)
    r = np.asarray(r, dtype=np.float32)
    u = np.asarray(u, dtype=np.float32)
    assert q.shape == (N,) and r.shape == (N, 3) and u.shape == (N, 3)

    if "nc" not in _NC_CACHE:
        _NC_CACHE["nc"] = _build_nc()
    nc = _NC_CACHE["nc"]

    in_maps, percore_host = _prep_inputs(q, r, u)
    if TRACE:
        _ensure_ntff_hook()
        # the NRT profile start needs a connected axon client: warm up untraced
        bass_utils.run_bass_kernel_spmd(
            nc, in_maps, core_ids=list(range(NCORES)), trace=False
        )
    results = bass_utils.run_bass_kernel_spmd(
        nc, in_maps, core_ids=list(range(NCORES)), trace=TRACE
    )
    LAST_EXEC_NS = results.exec_time_ns
    if results.instructions_and_trace is not None:
        LAST_TRACE = results.instructions_and_trace[1]

    total = 0.0
    for k in range(NCORES):
        total += _assemble(results.results[k], percore_host[k])
    pot = total * NORM / (4.0 * np.pi)
    return np.float32(pot)
